# revision 13
# baseline (speedup 1.0000x reference)
"""Bass/Trainium2 kernel for DropConnect (training path, Wstd != 0).

Z[b,o] = sum_i X[b,i] * W[i,o] * Werr[loc_id[b],i,o] + bias[o] * Berr[loc_id[b],o]

Strategy (8 NeuronCores, data-parallel over batch), v7:
  - each core handles 16 samples; the bf16 product pool V = bf16(W*Werr) is
    precomputed host-side (halves gather traffic vs f32 and removes the
    per-sample fp32 VectorE multiply that throttled v1); V viewed as
    macro-rows [128000, 2048], sample b's slab is rows loc*128..loc*128+127
    (partition p holds input rows i=4p..4p+3)
  - the 16 slab gathers are issued up front and spread over THREE DMA
    paths so the SDMA engines interleave packets from multiple rings (one
    ring alone sustains only ~295GB/s due to per-DMA completion gaps):
    8 on the Sync HWDGE queue and 4 on the Scalar HWDGE queue as plain
    direct DMAs with *register* row offsets (loc values values_load-ed
    once into SP/ACT sequencer registers; ts(loc,128) slices), and 4 via
    GpSimd indirect DMA
  - TensorE contracts each slab with bf16 X columns into a [1,512] PSUM
    tile - 4 matmuls per sample, nothing else, since the PE stream at
    ~300ns/matmul (LDWEIGHTS-serialized) is the secondary bottleneck
  - VectorE evacuates each PSUM row with a single tensor_add against the
    host-combined bias*Berr[loc] row (staged on partition 0), writing the
    [1, 8192] staging tile directly; this removes the 16 one-hot bias
    matmuls AND the 16 ScalarE copies of v3
  - the staging tile ships in four quarter DMAs on the Sync queue
"""

import sys

sys.path.insert(0, "/opt/trn_rl_repo")

import ml_dtypes
import numpy as np

B, IN, OUT, POOL, NCORES = 128, 512, 512, 1000, 8
BL = B // NCORES  # samples per core
WT_COLS = 4 * OUT  # 2048: one macro-row = 4 input rows of V
GP_SET = (8, 9, 10, 11)  # samples gathered via gpsimd indirect DMA
ACT_SET = (4, 5, 6, 7)  # samples gathered on the Scalar HWDGE queue

_CACHE = {}


def _build(pool_entries=POOL):
    import concourse.bass as bass
    import concourse.mybir as mybir
    import concourse.tile as tile
    from concourse import bacc
    from concourse.bass import ts

    f32, i32, bf16 = mybir.dt.float32, mybir.dt.int32, mybir.dt.bfloat16

    nc = bacc.Bacc("TRN2", debug=False)
    v = nc.dram_tensor("V", [pool_entries * 128, WT_COLS], bf16, kind="ExternalInput")
    xt = nc.dram_tensor("Xt", [128, BL * 4], bf16, kind="ExternalInput")
    idx = nc.dram_tensor("idx", [128, len(GP_SET)], i32, kind="ExternalInput")
    loc2 = nc.dram_tensor("loc2", [1, BL], i32, kind="ExternalInput")
    bbx = nc.dram_tensor("bbx", [1, BL * OUT], bf16, kind="ExternalInput")
    z = nc.dram_tensor("Z", [1, BL * OUT], f32, kind="ExternalOutput")

    with tile.TileContext(nc) as tc:
        with (
            tc.tile_pool(name="const", bufs=1) as cpool,
            tc.tile_pool(name="wts", bufs=1) as wpool,
            tc.tile_pool(name="ps", bufs=8, space="PSUM") as ppool,
        ):
            # index loads first: every gather is gated on one of these
            idx_sb = cpool.tile([128, len(GP_SET)], i32)
            nc.sync.dma_start(idx_sb[:], idx.ap())
            loc2_sb = cpool.tile([1, BL], i32)
            nc.scalar.dma_start(loc2_sb[:], loc2.ap())
            _, locv = nc.values_load_multi_w_load_instructions(
                loc2_sb[0:1, :],
                engines=[mybir.EngineType.SP, mybir.EngineType.Activation],
                skip_runtime_bounds_check=True,
            )
            xt_sb = cpool.tile([128, BL * 4], bf16)
            nc.scalar.dma_start(xt_sb[:], xt.ap())
            bbx_sb = cpool.tile([1, BL * OUT], bf16)
            nc.scalar.dma_start(bbx_sb[:], bbx.ap())
            zstage = cpool.tile([1, BL * OUT], f32)

            # phase 1: issue every slab gather, spread over three DMA rings
            slabs = []
            for b in range(BL):
                st = wpool.tile([128, WT_COLS], bf16, name=f"slab{b}")
                if b in GP_SET:
                    nc.gpsimd.indirect_dma_start(
                        out=st[:],
                        out_offset=None,
                        in_=v.ap(),
                        in_offset=bass.IndirectOffsetOnAxis(
                            ap=idx_sb[:, GP_SET.index(b) : GP_SET.index(b) + 1],
                            axis=0,
                        ),
                    )
                else:
                    eng = nc.scalar if b in ACT_SET else nc.sync
                    eng.dma_start(st[:], v.ap()[ts(locv[b], 128), :])
                slabs.append(st)

            # phase 2: matmuls + biased psum evacuation per sample
            for b in range(BL):
                ps = ppool.tile([1, OUT], f32, tag="ps")
                for j in range(4):
                    nc.tensor.matmul(
                        out=ps[:],
                        lhsT=xt_sb[:, 4 * b + j : 4 * b + j + 1],
                        rhs=slabs[b][:, j * OUT : (j + 1) * OUT],
                        start=(j == 0),
                        stop=(j == 3),
                    )
                nc.vector.tensor_add(
                    zstage[0:1, b * OUT : (b + 1) * OUT],
                    ps[:],
                    bbx_sb[0:1, b * OUT : (b + 1) * OUT],
                )
                if b % 4 == 3:
                    # ship each completed output quarter while later samples run
                    nc.sync.dma_start(
                        z.ap()[:, (b - 3) * OUT : (b + 1) * OUT],
                        zstage[0:1, (b - 3) * OUT : (b + 1) * OUT],
                    )

    nc.compile()
    return nc


def get_nc(pool_entries=POOL):
    key = ("nc", pool_entries)
    if key not in _CACHE:
        _CACHE[key] = _build(pool_entries)
    return _CACHE[key]


def make_in_maps(X, W, bias, Werr, Berr, loc_id):
    bf16 = ml_dtypes.bfloat16
    X = np.asarray(X, dtype=np.float32)
    W = np.asarray(W, dtype=np.float32)
    bias = np.asarray(bias, dtype=np.float32)
    Werr = np.asarray(Werr, dtype=np.float32)
    Berr = np.asarray(Berr, dtype=np.float32)
    loc_id = np.ascontiguousarray(np.asarray(loc_id, dtype=np.int32))

    pool_entries = Werr.shape[0]
    v2d = np.ascontiguousarray(
        (W[None, :, :] * Werr).reshape(pool_entries * 128, WT_COLS).astype(bf16)
    )
    p_iota = np.arange(128, dtype=np.int32)[:, None]

    in_maps = []
    for c in range(NCORES):
        xc = X[c * BL : (c + 1) * BL]  # [BL, IN]
        locc = loc_id[c * BL : (c + 1) * BL]  # [BL]
        xtc = np.ascontiguousarray(
            xc.reshape(BL, 128, 4).transpose(1, 0, 2).reshape(128, BL * 4).astype(bf16)
        )
        idxc = np.ascontiguousarray(
            locc[None, list(GP_SET)] * 128 + p_iota
        ).astype(np.int32)
        bbxc = (bias[None, :] * Berr[locc]).astype(bf16)
        in_maps.append(
            {
                "V": v2d,
                "Xt": xtc,
                "idx": idxc,
                "loc2": np.ascontiguousarray(locc[None, :]),
                "bbx": np.ascontiguousarray(bbxc.reshape(1, BL * OUT)),
            }
        )
    return in_maps


def _reset_accelerator():
    import ctypes

    try:
        lib = ctypes.CDLL("/opt/axon/libaxon_pjrt.so")
        lib.axon_reset.restype = ctypes.c_int64
        lib.axon_reset()
    except Exception:
        pass


def kernel(X, W, bias, Werr, Berr, loc_id):
    from concourse.bass_utils import run_bass_kernel_spmd

    nc = get_nc()
    in_maps = make_in_maps(X, W, bias, Werr, Berr, loc_id)
    try:
        res = run_bass_kernel_spmd(nc, in_maps, core_ids=list(range(NCORES)))
    except Exception:
        # a wedged NeuronCore surfaces as an unrecoverable-device error;
        # reset the accelerator once and retry
        _reset_accelerator()
        res = run_bass_kernel_spmd(nc, in_maps, core_ids=list(range(NCORES)))
    out = np.concatenate(
        [res.results[c]["Z"].reshape(BL, OUT) for c in range(NCORES)], axis=0
    )
    return np.ascontiguousarray(out, dtype=np.float32)


# revision 14
# speedup vs baseline: 1.0830x; 1.0830x over previous
"""Bass/Trainium2 kernel for DropConnect (training path, Wstd != 0).

Z[b,o] = sum_i X[b,i] * W[i,o] * Werr[loc_id[b],i,o] + bias[o] * Berr[loc_id[b],o]

Strategy (8 NeuronCores, data-parallel over batch), v8:
  - each core handles 16 samples; the bf16 product pool V = bf16(W*Werr) is
    precomputed host-side (halves gather traffic vs f32 and removes the
    per-sample fp32 VectorE multiply that throttled v1); V viewed as
    macro-rows [128000, 2048], sample b's slab is rows loc*128..loc*128+127
    (partition p holds input rows i=4p..4p+3)
  - the 16 slab gathers are issued up front and spread over THREE DMA
    paths so the SDMA engines interleave packets from multiple rings (one
    ring alone sustains only ~295GB/s due to per-DMA completion gaps):
    8 on the Sync HWDGE queue and 8 on the Scalar HWDGE queue as plain
    direct DMAs with *register* row offsets (loc values values_load-ed
    once into SP/ACT sequencer registers; ts(loc,128) slices); the GpSimd
    indirect path is avoided entirely - its inter-DMA ring DRAINs (~2us
    each) starve whichever slabs it serves
  - TensorE contracts each slab with bf16 X columns into a [1,512] PSUM
    tile - 4 matmuls per sample, nothing else, since the PE stream at
    ~300ns/matmul (LDWEIGHTS-serialized) is the secondary bottleneck
  - VectorE evacuates each PSUM row with a single tensor_add against the
    host-combined bias*Berr[loc] row (staged on partition 0), writing the
    [1, 8192] staging tile directly; this removes the 16 one-hot bias
    matmuls AND the 16 ScalarE copies of v3
  - the staging tile ships in four quarter DMAs on the Sync queue
"""

import sys

sys.path.insert(0, "/opt/trn_rl_repo")

import ml_dtypes
import numpy as np

B, IN, OUT, POOL, NCORES = 128, 512, 512, 1000, 8
BL = B // NCORES  # samples per core
WT_COLS = 4 * OUT  # 2048: one macro-row = 4 input rows of V
ACT_SET = (0, 1, 2, 3, 4, 5, 6, 7)  # samples gathered on the Scalar HWDGE queue

_CACHE = {}


def _build(pool_entries=POOL):
    import concourse.bass as bass
    import concourse.mybir as mybir
    import concourse.tile as tile
    from concourse import bacc
    from concourse.bass import ts

    f32, i32, bf16 = mybir.dt.float32, mybir.dt.int32, mybir.dt.bfloat16

    nc = bacc.Bacc("TRN2", debug=False)
    v = nc.dram_tensor("V", [pool_entries * 128, WT_COLS], bf16, kind="ExternalInput")
    xt = nc.dram_tensor("Xt", [128, BL * 4], bf16, kind="ExternalInput")
    loc2 = nc.dram_tensor("loc2", [1, BL], i32, kind="ExternalInput")
    bbx = nc.dram_tensor("bbx", [1, BL * OUT], bf16, kind="ExternalInput")
    z = nc.dram_tensor("Z", [1, BL * OUT], f32, kind="ExternalOutput")

    with tile.TileContext(nc) as tc:
        with (
            tc.tile_pool(name="const", bufs=1) as cpool,
            tc.tile_pool(name="wts", bufs=1) as wpool,
            tc.tile_pool(name="ps", bufs=8, space="PSUM") as ppool,
        ):
            # index load first: every gather is gated on it
            loc2_sb = cpool.tile([1, BL], i32)
            nc.scalar.dma_start(loc2_sb[:], loc2.ap())
            _, locv = nc.values_load_multi_w_load_instructions(
                loc2_sb[0:1, :],
                engines=[mybir.EngineType.SP, mybir.EngineType.Activation],
                skip_runtime_bounds_check=True,
            )
            xt_sb = cpool.tile([128, BL * 4], bf16)
            nc.scalar.dma_start(xt_sb[:], xt.ap())
            bbx_sb = cpool.tile([1, BL * OUT], bf16)
            nc.scalar.dma_start(bbx_sb[:], bbx.ap())
            zstage = cpool.tile([1, BL * OUT], f32)

            # phase 1: issue every slab gather, spread over three DMA rings
            slabs = []
            for b in range(BL):
                st = wpool.tile([128, WT_COLS], bf16, name=f"slab{b}")
                eng = nc.scalar if b in ACT_SET else nc.sync
                eng.dma_start(st[:], v.ap()[ts(locv[b], 128), :])
                slabs.append(st)

            # phase 2: matmuls + biased psum evacuation per sample
            for b in range(BL):
                ps = ppool.tile([1, OUT], f32, tag="ps")
                for j in range(4):
                    nc.tensor.matmul(
                        out=ps[:],
                        lhsT=xt_sb[:, 4 * b + j : 4 * b + j + 1],
                        rhs=slabs[b][:, j * OUT : (j + 1) * OUT],
                        start=(j == 0),
                        stop=(j == 3),
                    )
                nc.vector.tensor_add(
                    zstage[0:1, b * OUT : (b + 1) * OUT],
                    ps[:],
                    bbx_sb[0:1, b * OUT : (b + 1) * OUT],
                )
                if b % 4 == 3:
                    # ship each completed output quarter while later samples run
                    nc.sync.dma_start(
                        z.ap()[:, (b - 3) * OUT : (b + 1) * OUT],
                        zstage[0:1, (b - 3) * OUT : (b + 1) * OUT],
                    )

    nc.compile()
    return nc


def get_nc(pool_entries=POOL):
    key = ("nc", pool_entries)
    if key not in _CACHE:
        _CACHE[key] = _build(pool_entries)
    return _CACHE[key]


def make_in_maps(X, W, bias, Werr, Berr, loc_id):
    bf16 = ml_dtypes.bfloat16
    X = np.asarray(X, dtype=np.float32)
    W = np.asarray(W, dtype=np.float32)
    bias = np.asarray(bias, dtype=np.float32)
    Werr = np.asarray(Werr, dtype=np.float32)
    Berr = np.asarray(Berr, dtype=np.float32)
    loc_id = np.ascontiguousarray(np.asarray(loc_id, dtype=np.int32))

    pool_entries = Werr.shape[0]
    v2d = np.ascontiguousarray(
        (W[None, :, :] * Werr).reshape(pool_entries * 128, WT_COLS).astype(bf16)
    )
    in_maps = []
    for c in range(NCORES):
        xc = X[c * BL : (c + 1) * BL]  # [BL, IN]
        locc = loc_id[c * BL : (c + 1) * BL]  # [BL]
        xtc = np.ascontiguousarray(
            xc.reshape(BL, 128, 4).transpose(1, 0, 2).reshape(128, BL * 4).astype(bf16)
        )
        bbxc = (bias[None, :] * Berr[locc]).astype(bf16)
        in_maps.append(
            {
                "V": v2d,
                "Xt": xtc,
                "loc2": np.ascontiguousarray(locc[None, :]),
                "bbx": np.ascontiguousarray(bbxc.reshape(1, BL * OUT)),
            }
        )
    return in_maps


def _reset_accelerator():
    import ctypes

    try:
        lib = ctypes.CDLL("/opt/axon/libaxon_pjrt.so")
        lib.axon_reset.restype = ctypes.c_int64
        lib.axon_reset()
    except Exception:
        pass


def kernel(X, W, bias, Werr, Berr, loc_id):
    from concourse.bass_utils import run_bass_kernel_spmd

    nc = get_nc()
    in_maps = make_in_maps(X, W, bias, Werr, Berr, loc_id)
    try:
        res = run_bass_kernel_spmd(nc, in_maps, core_ids=list(range(NCORES)))
    except Exception:
        # a wedged NeuronCore surfaces as an unrecoverable-device error;
        # reset the accelerator once and retry
        _reset_accelerator()
        res = run_bass_kernel_spmd(nc, in_maps, core_ids=list(range(NCORES)))
    out = np.concatenate(
        [res.results[c]["Z"].reshape(BL, OUT) for c in range(NCORES)], axis=0
    )
    return np.ascontiguousarray(out, dtype=np.float32)
